# revision 19
# baseline (speedup 1.0000x reference)
"""EncDec ConvLSTM kernel for 8 Trainium2 NeuronCores.

Sharding: 8 cores = 4 (batch) x 2 (spatial row-halves). Each core computes
its 32 output rows plus a shrinking redundant halo (21-s extra rows at
recurrent step s), so no cross-core communication is needed. Row-half 1
cores receive a vertically flipped image and ky-flipped conv weights, so a
single SPMD program serves all cores.

Conv3x3 is mapped to PE matmuls over pixels (N=512 free dim, fp32r):
per 8-row tile the 4H=256 gate channels come from 2 M-tiles x 7
accumulating matmuls (1 x-im2col K=72 + 3 paired h-taps K=128 + 3 single
h-taps K=64). The kx=0/kx=2 h-taps are packed into one K=128 matmul using
a column-shifted copy of h kept in partitions 64..127.
"""

import os
import sys

import numpy as np

for _p in ("/opt/trn_rl_repo", "/root/.axon_site/_ro/trn_rl_repo"):
    if os.path.isdir(_p) and _p not in sys.path:
        sys.path.append(_p)

T = 10
F = 8
HD = 64
HS = 64
WS = 64
NCORES = 8
PW = 66  # padded grid width/height
NSTEPS = 2 * T

_CACHE = {}


def _regions():
    """Rounded compute-region row counts per recurrent step s=1..NSTEPS."""
    out = []
    for s in range(1, NSTEPS + 1):
        need = NSTEPS + 1 - s
        rows = min(HS, 32 + need)
        rows = min(HS, ((rows + 7) // 8) * 8)
        out.append(rows)
    return out


def _build_program(use_bf16=True):
    from concourse import bacc, mybir, tile

    F32 = mybir.dt.float32
    MMDT = mybir.dt.bfloat16 if use_bf16 else mybir.dt.float32r
    ACT = mybir.ActivationFunctionType

    nc = bacc.Bacc("TRN2", target_bir_lowering=False, debug=False,
                   num_devices=NCORES)

    def din(name, shape, dt=MMDT):
        return nc.dram_tensor(name, shape, dt, kind="ExternalInput").ap()

    xe_d = din("xe", [T, F, PW, PW])
    xd_d = din("xd", [T, F, PW, PW])
    w_x = {"e": din("w_ex", [72, 256]), "d": din("w_dx", [72, 256])}
    w_p = {ph: [din(f"w_{ph}p{k}", [128, 256]) for k in range(3)]
           for ph in ("e", "d")}
    w_s = {ph: [din(f"w_{ph}s{k}", [64, 256]) for k in range(3)]
           for ph in ("e", "d")}
    w_op = [din(f"w_op{k}", [128, 8]) for k in range(3)]
    w_os = [din(f"w_os{k}", [64, 8]) for k in range(3)]
    b_m0 = {"e": din("b_e0", [128, 1], F32), "d": din("b_d0", [128, 1], F32)}
    b_m1 = {"e": din("b_e1", [128, 1], F32), "d": din("b_d1", [128, 1], F32)}
    b_o = din("b_o", [8, 1], F32)
    zz_d = din("zz", [128, PW * PW])  # fp32r zeros for state init
    y_d = nc.dram_tensor("y", [T, F, 32, WS], F32, kind="ExternalOutput").ap()

    regions = _regions()

    with tile.TileContext(nc) as tc:
        with tc.tile_pool(name="wpool", bufs=1) as wp, \
             tc.tile_pool(name="state", bufs=1) as stp, \
             tc.tile_pool(name="x2p", bufs=2) as x2p, \
             tc.tile_pool(name="gps", bufs=3, space="PSUM") as gps, \
             tc.tile_pool(name="ops", bufs=2, space="PSUM") as ops, \
             tc.tile_pool(name="fip", bufs=3) as fip, \
             tc.tile_pool(name="ogp", bufs=3) as ogp, \
             tc.tile_pool(name="t1p", bufs=3) as t1p, \
             tc.tile_pool(name="t1lp", bufs=3) as t1lp, \
             tc.tile_pool(name="thp", bufs=3) as thp, \
             tc.tile_pool(name="yyp", bufs=2) as yyp:

            # ---- load weights / biases into SBUF ----
            def wtile(src, shape, tag, dt=MMDT):
                t_ = wp.tile(shape, dt, tag=tag)
                nc.sync.dma_start(t_[:], src[:])
                return t_

            sw_x = {ph: wtile(w_x[ph], [72, 256], f"wx{ph}")
                    for ph in ("e", "d")}
            sw_p = {ph: [wtile(w_p[ph][k], [128, 256], f"wp{ph}{k}")
                         for k in range(3)] for ph in ("e", "d")}
            sw_s = {ph: [wtile(w_s[ph][k], [64, 256], f"ws{ph}{k}")
                         for k in range(3)] for ph in ("e", "d")}
            sw_op = [wtile(w_op[k], [128, 8], f"wop{k}") for k in range(3)]
            sw_os = [wtile(w_os[k], [64, 8], f"wos{k}") for k in range(3)]
            sb_m0 = {ph: wtile(b_m0[ph], [128, 1], f"b0{ph}", F32)
                     for ph in ("e", "d")}
            sb_m1 = {ph: wtile(b_m1[ph], [128, 1], f"b1{ph}", F32)
                     for ph in ("e", "d")}
            sb_o = wtile(b_o, [8, 1], "bo", F32)

            # ---- persistent state ----
            hhA = stp.tile([128, PW * PW], MMDT, tag="hhA")
            hhB = stp.tile([128, PW * PW], MMDT, tag="hhB")
            c_t = stp.tile([64, PW * PW], F32, tag="c")
            nc.sync.dma_start(hhA[:], zz_d[:])
            nc.sync.dma_start(hhB[:], zz_d[:])
            nc.vector.memset(c_t[:], 0.0)

            # PE clock warm-up: a dense run of small-weight matmuls keeps
            # the PE array near-100% active so HAM raises the clock to
            # 2.4GHz before the real work starts. Gate matmuls alone never
            # warm it (128-col LDWEIGHTS between every MM lowers the
            # array's duty cycle below HAM's busy threshold).
            for _ in range(64):
                wu = ops.tile([8, 512], F32, tag="pso")
                nc.tensor.matmul(wu[:], sw_op[0][:], hhA[:, 0:512],
                                 start=True, stop=True)
            hhAv = hhA[:].rearrange("p (r c) -> p r c", c=PW)
            hhBv = hhB[:].rearrange("p (r c) -> p r c", c=PW)
            c_v = c_t[:].rearrange("p (r c) -> p r c", c=PW)

            def emit_x2col(s):
                """Load x im2col for step s: partition (ky*3+kx)*8+ic holds
                the flat padded image shifted by ky*66+kx (contiguous)."""
                ph = "e" if s <= T else "d"
                t_idx = (s - 1) if ph == "e" else (s - 1 - T)
                x_src = xe_d if ph == "e" else xd_d
                rp = regions[s - 1]
                ln = (rp - 1) * PW + 64
                x2 = x2p.tile([72, 57 * PW], MMDT, tag="x2")
                flat = x_src[t_idx].rearrange("a r c -> a (r c)")
                for tap in range(9):
                    sh = (tap // 3) * PW + (tap % 3)
                    nc.gpsimd.dma_start(x2[tap * 8:(tap + 1) * 8, 0:ln],
                                        flat[:, sh:sh + ln])
                return x2

            def emit_outconv(s, h_view):
                """relu(out conv) for decoder step s, reading its h buffer."""
                t_o = s - 1 - T
                for n2 in range(4):
                    r0 = n2 * 8
                    pso = ops.tile([8, 512], F32, tag="pso")
                    for k in range(3):
                        nc.tensor.matmul(pso[:], sw_op[k][:],
                                         h_view[:, r0 + k:r0 + k + 8, 0:64],
                                         start=(k == 0), stop=False)
                    for k in range(3):
                        nc.tensor.matmul(pso[:], sw_os[k][:],
                                         h_view[0:64, r0 + k:r0 + k + 8, 1:65],
                                         start=False, stop=(k == 2))
                    yy = yyp.tile([8, 512], F32, tag="yy")
                    nc.scalar.activation(yy[:], pso[:], ACT.Relu,
                                         bias=sb_o[:])
                    nc.gpsimd.dma_start(
                        y_d[t_o, :, r0:r0 + 8, :],
                        yy[:].rearrange("p (r c) -> p r c", c=64))

            x2_cur = emit_x2col(1)
            for s in range(1, NSTEPS + 1):
                ph = "e" if s <= T else "d"
                rp = regions[s - 1]
                ntiles = rp // 8
                h_r = hhAv if (s % 2 == 0) else hhBv  # read: written at s-1
                h_w = hhBv if (s % 2 == 0) else hhAv

                if s > T + 1:
                    emit_outconv(s - 1, h_r)  # prev decoder step, deps long resolved
                x2v = x2_cur[:].rearrange("p (r c) -> p r c", c=PW)
                if s < NSTEPS:
                    x2_next = emit_x2col(s + 1)  # prefetch on gpsimd queue

                for n0 in range(0, ntiles, 2):
                    npair = 2 if n0 + 1 < ntiles else 1
                    w = 512 * npair
                    rows = 8 * npair
                    r0 = n0 * 8
                    ps0 = gps.tile([128, 1024], F32, tag="ps")
                    ps1 = gps.tile([128, 1024], F32, tag="ps")
                    for m, ps in ((0, ps0), (1, ps1)):
                        ms = slice(m * 128, (m + 1) * 128)
                        for sub in range(npair):
                            rs = r0 + sub * 8
                            pv = ps[:, sub * 512:(sub + 1) * 512]
                            nc.tensor.matmul(pv, sw_x[ph][:, ms],
                                             x2v[0:72, rs:rs + 8, 0:64],
                                             start=True, stop=False)
                            for k in range(3):
                                nc.tensor.matmul(
                                    pv, sw_p[ph][k][:, ms],
                                    h_r[:, rs + k:rs + k + 8, 0:64],
                                    start=False, stop=False)
                            for k in range(3):
                                nc.tensor.matmul(
                                    pv, sw_s[ph][k][:, ms],
                                    h_r[0:64, rs + k:rs + k + 8, 1:65],
                                    start=False, stop=(k == 2))

                    # epilogue: M0=[f;i] M1=[o;g]
                    fi = fip.tile([128, 1024], F32, tag="fi")
                    og = ogp.tile([128, 1024], F32, tag="og")
                    nc.scalar.activation(fi[:, 0:w], ps0[:, 0:w], ACT.Sigmoid,
                                         bias=sb_m0[ph][:])
                    nc.scalar.activation(og[0:64, 0:w], ps1[0:64, 0:w],
                                         ACT.Sigmoid, bias=sb_m1[ph][0:64])
                    nc.scalar.activation(og[64:128, 0:w], ps1[64:128, 0:w],
                                         ACT.Tanh, bias=sb_m1[ph][64:128])
                    # t1 = sigmoid(i) * tanh(g) on partitions 64..127
                    t1 = t1p.tile([128, 1024], F32, tag="t1")
                    nc.vector.tensor_mul(t1[64:128, 0:w], fi[64:128, 0:w],
                                         og[64:128, 0:w])
                    # cross-partition move 64..127 -> 0..63
                    t1l = t1lp.tile([64, 1024], F32, tag="t1l")
                    nc.sync.dma_start(t1l[:, 0:w], t1[64:128, 0:w])
                    t1lv = t1l[:].rearrange("p (r c) -> p r c", c=64)
                    fiv = fi[0:64].rearrange("p (r c) -> p r c", c=64)
                    ogv = og[0:64].rearrange("p (r c) -> p r c", c=64)
                    cs = c_v[0:64, r0 + 1:r0 + 1 + rows, 1:65]
                    nc.vector.tensor_mul(cs, cs, fiv[:, 0:rows, :])
                    nc.vector.tensor_add(cs, cs, t1lv[:, 0:rows, :])
                    th = thp.tile([64, 1024], F32, tag="th")
                    thv = th[:].rearrange("p (r c) -> p r c", c=64)
                    nc.scalar.activation(thv[:, 0:rows, :], cs, ACT.Tanh)
                    # h = tanh(c) * sigmoid(o) -> base half of write buffer
                    nc.vector.tensor_mul(
                        h_w[0:64, r0 + 1:r0 + 1 + rows, 1:65],
                        thv[:, 0:rows, :], ogv[:, 0:rows, :])
                    # shifted copy (cols +2) into partitions 64..127
                    nc.sync.dma_start(
                        h_w[64:128, r0 + 1:r0 + 1 + rows, 0:64],
                        h_w[0:64, r0 + 1:r0 + 1 + rows, 2:66])

                if s < NSTEPS:
                    x2_cur = x2_next

            # out conv for the final decoder step
            emit_outconv(NSTEPS, hhBv if NSTEPS % 2 == 0 else hhAv)

    nc.compile()
    return nc


def _prep_core_inputs(core, enc_in, dec_in, enc_W, enc_b, dec_W, dec_b,
                      out_W, out_b, use_bf16=True):
    import ml_dtypes
    mm_np = ml_dtypes.bfloat16 if use_bf16 else np.float32
    b, half = core // 2, core % 2
    # gate permutation: [f, i, o, g]
    perm = np.concatenate([np.arange(0, 128), np.arange(192, 256),
                           np.arange(128, 192)])

    def prep_x(x):
        x = x[b]  # [T, F, 64, 64]
        if half:
            x = x[:, :, ::-1, :]
        xp = np.zeros((T, F, PW, PW), np.float32)
        xp[:, :, 1:65, 1:65] = x
        return np.ascontiguousarray(xp)

    def prep_gateW(W, bias):
        Wf = W[:, :, ::-1, :] if half else W
        Wp = np.ascontiguousarray(Wf[perm])  # [256, 72, 3, 3]
        bp = bias[perm].astype(np.float32)
        # x part: rows (ky*3+kx)*8+ic
        lx = Wp[:, :F].transpose(2, 3, 1, 0).reshape(72, 256)
        lp = [np.concatenate([Wp[:, F:, k, 0].T, Wp[:, F:, k, 2].T], axis=0)
              for k in range(3)]  # [128, 256]
        ls = [np.ascontiguousarray(Wp[:, F:, k, 1].T) for k in range(3)]
        return (np.ascontiguousarray(lx),
                [np.ascontiguousarray(a) for a in lp], ls,
                np.ascontiguousarray(bp[0:128].reshape(128, 1)),
                np.ascontiguousarray(bp[128:256].reshape(128, 1)))

    ex, ep, es, eb0, eb1 = prep_gateW(enc_W, enc_b)
    dx, dp, ds, db0, db1 = prep_gateW(dec_W, dec_b)
    oWf = out_W[:, :, ::-1, :] if half else out_W
    op = [np.ascontiguousarray(np.concatenate(
        [oWf[:, :, k, 0].T, oWf[:, :, k, 2].T], axis=0).astype(np.float32))
        for k in range(3)]
    osng = [np.ascontiguousarray(oWf[:, :, k, 1].T.astype(np.float32))
            for k in range(3)]

    m = {"xe": prep_x(enc_in), "xd": prep_x(dec_in),
         "w_ex": ex, "w_dx": dx,
         "b_e0": eb0, "b_e1": eb1, "b_d0": db0, "b_d1": db1,
         "b_o": np.ascontiguousarray(out_b.reshape(8, 1).astype(np.float32)),
         "zz": np.zeros((128, PW * PW), np.float32)}
    for k in range(3):
        m[f"w_ep{k}"] = ep[k]
        m[f"w_es{k}"] = es[k]
        m[f"w_dp{k}"] = dp[k]
        m[f"w_ds{k}"] = ds[k]
        m[f"w_op{k}"] = op[k]
        m[f"w_os{k}"] = osng[k]
    f32_keys = {"b_e0", "b_e1", "b_d0", "b_d1", "b_o"}
    return {k: np.ascontiguousarray(np.asarray(
        v, np.float32 if k in f32_keys else mm_np)) for k, v in m.items()}


def _install_trace_hook():
    """Shim antenv.axon_hooks for NTFF profiling (dev only)."""
    import contextlib
    import ctypes
    import types

    so = "/opt/axon/libaxon_pjrt.so"
    if "antenv.axon_hooks" in sys.modules or not os.path.exists(so):
        return
    lib = ctypes.CDLL(so)
    if not hasattr(lib, "axon_start_nrt_profile"):
        return
    lib.axon_start_nrt_profile.argtypes = [ctypes.POINTER(ctypes.c_int64),
                                           ctypes.c_size_t]
    lib.axon_start_nrt_profile.restype = ctypes.c_int64
    lib.axon_stop_nrt_profile.argtypes = [ctypes.c_char_p]
    lib.axon_stop_nrt_profile.restype = ctypes.c_int64

    def _mk():
        @contextlib.contextmanager
        def _hook(output_dir, device_ids):
            import jax
            jax.devices()
            if device_ids:
                ids = (ctypes.c_int64 * len(device_ids))(*device_ids)
                rc = lib.axon_start_nrt_profile(ids, len(device_ids))
            else:
                rc = lib.axon_start_nrt_profile(None, 0)
            if rc != 0:
                raise RuntimeError(f"axon_start_nrt_profile rc={rc}")
            try:
                yield
            finally:
                lib.axon_stop_nrt_profile(str(output_dir).encode())
        return _hook

    mod = types.ModuleType("antenv.axon_hooks")
    mod.get_axon_ntff_profile_hook = _mk
    sys.modules["antenv.axon_hooks"] = mod


def kernel(enc_in, dec_in, enc_W, enc_b, dec_W, dec_b, out_W, out_b):
    from concourse.bass_utils import run_bass_kernel_spmd

    trace = os.environ.get("KERNEL_TRACE", "") == "1"
    if trace:
        _install_trace_hook()

    use_bf16 = os.environ.get("KERNEL_DTYPE", "bf16") != "f32r"
    if "nc" not in _CACHE:
        _CACHE["nc"] = _build_program(use_bf16)
    nc = _CACHE["nc"]

    args = (np.asarray(enc_in, np.float32), np.asarray(dec_in, np.float32),
            np.asarray(enc_W, np.float32), np.asarray(enc_b, np.float32),
            np.asarray(dec_W, np.float32), np.asarray(dec_b, np.float32),
            np.asarray(out_W, np.float32), np.asarray(out_b, np.float32))
    in_maps = [_prep_core_inputs(c, *args, use_bf16=use_bf16)
               for c in range(NCORES)]

    res = run_bass_kernel_spmd(nc, in_maps, list(range(NCORES)), trace=trace)
    if trace:
        _CACHE["exec_time_ns"] = res.exec_time_ns

    B = enc_in.shape[0]
    out = np.empty((B, T, F, HS, WS), np.float32)
    for c in range(NCORES):
        b, half = c // 2, c % 2
        yc = res.results[c]["y"]  # [T, F, 32, 64]
        if half:
            out[b, :, :, 32:64, :] = yc[:, :, ::-1, :]
        else:
            out[b, :, :, 0:32, :] = yc
    return out


# revision 21
# speedup vs baseline: 1.0184x; 1.0184x over previous
"""EncDec ConvLSTM kernel for 8 Trainium2 NeuronCores.

Sharding: 8 cores = 4 (batch) x 2 (spatial row-halves). Each core computes
its 32 output rows plus a shrinking redundant halo (21-s extra rows at
recurrent step s), so no cross-core communication is needed. Row-half 1
cores receive a vertically flipped image and ky-flipped conv weights, so a
single SPMD program serves all cores.

Conv3x3 is mapped to PE matmuls over pixels (N=512 free dim, fp32r):
per 8-row tile the 4H=256 gate channels come from 2 M-tiles x 7
accumulating matmuls (1 x-im2col K=72 + 3 paired h-taps K=128 + 3 single
h-taps K=64). The kx=0/kx=2 h-taps are packed into one K=128 matmul using
a column-shifted copy of h kept in partitions 64..127.
"""

import os
import sys

import numpy as np

for _p in ("/opt/trn_rl_repo", "/root/.axon_site/_ro/trn_rl_repo"):
    if os.path.isdir(_p) and _p not in sys.path:
        sys.path.append(_p)

T = 10
F = 8
HD = 64
HS = 64
WS = 64
NCORES = 8
PW = 66  # padded grid width/height
NSTEPS = 2 * T

_CACHE = {}


def _regions():
    """Rounded compute-region row counts per recurrent step s=1..NSTEPS."""
    out = []
    for s in range(1, NSTEPS + 1):
        need = NSTEPS + 1 - s
        rows = min(HS, 32 + need)
        rows = min(HS, ((rows + 7) // 8) * 8)
        out.append(rows)
    return out


def _build_program(use_bf16=True):
    from concourse import bacc, mybir, tile

    F32 = mybir.dt.float32
    MMDT = mybir.dt.bfloat16 if use_bf16 else mybir.dt.float32r
    ACT = mybir.ActivationFunctionType

    nc = bacc.Bacc("TRN2", target_bir_lowering=False, debug=False,
                   num_devices=NCORES)

    def din(name, shape, dt=MMDT):
        return nc.dram_tensor(name, shape, dt, kind="ExternalInput").ap()

    xe_d = din("xe", [T, F, PW, PW])
    xd_d = din("xd", [T, F, PW, PW])
    w_x = {"e": din("w_ex", [72, 256]), "d": din("w_dx", [72, 256])}
    w_p = {ph: [din(f"w_{ph}p{k}", [128, 256]) for k in range(3)]
           for ph in ("e", "d")}
    w_s = {ph: [din(f"w_{ph}s{k}", [64, 256]) for k in range(3)]
           for ph in ("e", "d")}
    w_op = [din(f"w_op{k}", [128, 8]) for k in range(3)]
    w_os = [din(f"w_os{k}", [64, 8]) for k in range(3)]
    b_m0 = {"e": din("b_e0", [128, 1], F32), "d": din("b_d0", [128, 1], F32)}
    b_m1 = {"e": din("b_e1", [128, 1], F32), "d": din("b_d1", [128, 1], F32)}
    b_o = din("b_o", [8, 1], F32)
    zz_d = din("zz", [128, PW * PW])  # fp32r zeros for state init
    y_d = nc.dram_tensor("y", [T, F, 32, WS], F32, kind="ExternalOutput").ap()

    regions = _regions()

    with tile.TileContext(nc) as tc:
        with tc.tile_pool(name="wpool", bufs=1) as wp, \
             tc.tile_pool(name="state", bufs=1) as stp, \
             tc.tile_pool(name="x2p", bufs=2) as x2p, \
             tc.tile_pool(name="gps", bufs=6, space="PSUM") as gps, \
             tc.tile_pool(name="ops", bufs=2, space="PSUM") as ops, \
             tc.tile_pool(name="fip", bufs=3) as fip, \
             tc.tile_pool(name="ogp", bufs=3) as ogp, \
             tc.tile_pool(name="t1p", bufs=3) as t1p, \
             tc.tile_pool(name="t1lp", bufs=3) as t1lp, \
             tc.tile_pool(name="thp", bufs=3) as thp, \
             tc.tile_pool(name="yyp", bufs=2) as yyp:

            # ---- load weights / biases into SBUF ----
            def wtile(src, shape, tag, dt=MMDT):
                t_ = wp.tile(shape, dt, tag=tag)
                nc.sync.dma_start(t_[:], src[:])
                return t_

            sw_x = {ph: wtile(w_x[ph], [72, 256], f"wx{ph}")
                    for ph in ("e", "d")}
            sw_p = {ph: [wtile(w_p[ph][k], [128, 256], f"wp{ph}{k}")
                         for k in range(3)] for ph in ("e", "d")}
            sw_s = {ph: [wtile(w_s[ph][k], [64, 256], f"ws{ph}{k}")
                         for k in range(3)] for ph in ("e", "d")}
            sw_op = [wtile(w_op[k], [128, 8], f"wop{k}") for k in range(3)]
            sw_os = [wtile(w_os[k], [64, 8], f"wos{k}") for k in range(3)]
            sb_m0 = {ph: wtile(b_m0[ph], [128, 1], f"b0{ph}", F32)
                     for ph in ("e", "d")}
            sb_m1 = {ph: wtile(b_m1[ph], [128, 1], f"b1{ph}", F32)
                     for ph in ("e", "d")}
            sb_o = wtile(b_o, [8, 1], "bo", F32)

            # ---- persistent state ----
            hhA = stp.tile([128, PW * PW], MMDT, tag="hhA")
            hhB = stp.tile([128, PW * PW], MMDT, tag="hhB")
            c_t = stp.tile([64, PW * PW], F32, tag="c")
            nc.sync.dma_start(hhA[:], zz_d[:])
            nc.sync.dma_start(hhB[:], zz_d[:])
            nc.vector.memset(c_t[:], 0.0)

            # PE clock warm-up: a dense run of small-weight matmuls keeps
            # the PE array near-100% active so HAM raises the clock to
            # 2.4GHz before the real work starts. Gate matmuls alone never
            # warm it (128-col LDWEIGHTS between every MM lowers the
            # array's duty cycle below HAM's busy threshold).
            for _ in range(64):
                wu = ops.tile([8, 512], F32, tag="pso")
                nc.tensor.matmul(wu[:], sw_op[0][:], hhA[:, 0:512],
                                 start=True, stop=True)
            hhAv = hhA[:].rearrange("p (r c) -> p r c", c=PW)
            hhBv = hhB[:].rearrange("p (r c) -> p r c", c=PW)
            c_v = c_t[:].rearrange("p (r c) -> p r c", c=PW)

            def emit_x2col(s):
                """Load x im2col for step s: partition (ky*3+kx)*8+ic holds
                the flat padded image shifted by ky*66+kx (contiguous)."""
                ph = "e" if s <= T else "d"
                t_idx = (s - 1) if ph == "e" else (s - 1 - T)
                x_src = xe_d if ph == "e" else xd_d
                rp = regions[s - 1]
                ln = (rp - 1) * PW + 64
                x2 = x2p.tile([72, 57 * PW], MMDT, tag="x2")
                flat = x_src[t_idx].rearrange("a r c -> a (r c)")
                for tap in range(9):
                    sh = (tap // 3) * PW + (tap % 3)
                    nc.gpsimd.dma_start(x2[tap * 8:(tap + 1) * 8, 0:ln],
                                        flat[:, sh:sh + ln])
                return x2

            def emit_outconv(s, h_view):
                """relu(out conv) for decoder step s, reading its h buffer."""
                t_o = s - 1 - T
                for n2 in range(4):
                    r0 = n2 * 8
                    pso = ops.tile([8, 512], F32, tag="pso")
                    for k in range(3):
                        nc.tensor.matmul(pso[:], sw_op[k][:],
                                         h_view[:, r0 + k:r0 + k + 8, 0:64],
                                         start=(k == 0), stop=False)
                    for k in range(3):
                        nc.tensor.matmul(pso[:], sw_os[k][:],
                                         h_view[0:64, r0 + k:r0 + k + 8, 1:65],
                                         start=False, stop=(k == 2))
                    yy = yyp.tile([8, 512], F32, tag="yy")
                    nc.scalar.activation(yy[:], pso[:], ACT.Relu,
                                         bias=sb_o[:])
                    nc.gpsimd.dma_start(
                        y_d[t_o, :, r0:r0 + 8, :],
                        yy[:].rearrange("p (r c) -> p r c", c=64))

            x2_cur = emit_x2col(1)
            for s in range(1, NSTEPS + 1):
                ph = "e" if s <= T else "d"
                rp = regions[s - 1]
                ntiles = rp // 8
                h_r = hhAv if (s % 2 == 0) else hhBv  # read: written at s-1
                h_w = hhBv if (s % 2 == 0) else hhAv

                if s > T + 1:
                    emit_outconv(s - 1, h_r)  # prev decoder step, deps long resolved
                x2v = x2_cur[:].rearrange("p (r c) -> p r c", c=PW)
                if s < NSTEPS:
                    x2_next = emit_x2col(s + 1)  # prefetch on gpsimd queue

                if 1 < s <= T:
                    # re-warm burst: dense small-weight matmuls with no data
                    # deps (read-only weight tiles) recover the PE clock if
                    # a pipeline bubble re-throttled it. Decoder steps get
                    # this for free from the out-conv blocks.
                    for _ in range(12):
                        wu = ops.tile([8, 512], F32, tag="pso")
                        nc.tensor.matmul(wu[:, 0:256], sw_os[0][:],
                                         sw_p["e"][0][0:64, 0:256],
                                         start=True, stop=True)

                for n in range(ntiles):
                    r0 = n * 8
                    ps0 = gps.tile([128, 512], F32, tag="ps")
                    ps1 = gps.tile([128, 512], F32, tag="ps")
                    for m, ps in ((0, ps0), (1, ps1)):
                        ms = slice(m * 128, (m + 1) * 128)
                        nc.tensor.matmul(ps[:], sw_x[ph][:, ms],
                                         x2v[0:72, r0:r0 + 8, 0:64],
                                         start=True, stop=False)
                        for k in range(3):
                            nc.tensor.matmul(
                                ps[:], sw_p[ph][k][:, ms],
                                h_r[:, r0 + k:r0 + k + 8, 0:64],
                                start=False, stop=False)
                        for k in range(3):
                            nc.tensor.matmul(
                                ps[:], sw_s[ph][k][:, ms],
                                h_r[0:64, r0 + k:r0 + k + 8, 1:65],
                                start=False, stop=(k == 2))

                    # epilogue: M0=[f;i] M1=[o;g]
                    fi = fip.tile([128, 512], F32, tag="fi")
                    og = ogp.tile([128, 512], F32, tag="og")
                    nc.scalar.activation(fi[:], ps0[:], ACT.Sigmoid,
                                         bias=sb_m0[ph][:])
                    nc.scalar.activation(og[0:64], ps1[0:64], ACT.Sigmoid,
                                         bias=sb_m1[ph][0:64])
                    nc.scalar.activation(og[64:128], ps1[64:128], ACT.Tanh,
                                         bias=sb_m1[ph][64:128])
                    # t1 = sigmoid(i) * tanh(g) on partitions 64..127
                    t1 = t1p.tile([128, 512], F32, tag="t1")
                    nc.vector.tensor_mul(t1[64:128], fi[64:128], og[64:128])
                    # cross-partition move 64..127 -> 0..63
                    t1l = t1lp.tile([64, 512], F32, tag="t1l")
                    nc.sync.dma_start(t1l[:], t1[64:128])
                    t1lv = t1l[:].rearrange("p (r c) -> p r c", c=64)
                    cs = c_v[0:64, r0 + 1:r0 + 9, 1:65]
                    nc.vector.tensor_mul(cs, cs, fi[0:64].rearrange(
                        "p (r c) -> p r c", c=64))
                    nc.vector.tensor_add(cs, cs, t1lv)
                    th = thp.tile([64, 512], F32, tag="th")
                    thv = th[:].rearrange("p (r c) -> p r c", c=64)
                    nc.scalar.activation(thv, cs, ACT.Tanh)
                    # h = tanh(c) * sigmoid(o) -> base half of write buffer
                    nc.vector.tensor_mul(
                        h_w[0:64, r0 + 1:r0 + 9, 1:65], thv,
                        og[0:64].rearrange("p (r c) -> p r c", c=64))
                    # shifted copy (cols +2) into partitions 64..127
                    nc.sync.dma_start(
                        h_w[64:128, r0 + 1:r0 + 9, 0:64],
                        h_w[0:64, r0 + 1:r0 + 9, 2:66])

                if s < NSTEPS:
                    x2_cur = x2_next

            # out conv for the final decoder step
            emit_outconv(NSTEPS, hhBv if NSTEPS % 2 == 0 else hhAv)

    nc.compile()
    return nc


def _prep_core_inputs(core, enc_in, dec_in, enc_W, enc_b, dec_W, dec_b,
                      out_W, out_b, use_bf16=True):
    import ml_dtypes
    mm_np = ml_dtypes.bfloat16 if use_bf16 else np.float32
    b, half = core // 2, core % 2
    # gate permutation: [f, i, o, g]
    perm = np.concatenate([np.arange(0, 128), np.arange(192, 256),
                           np.arange(128, 192)])

    def prep_x(x):
        x = x[b]  # [T, F, 64, 64]
        if half:
            x = x[:, :, ::-1, :]
        xp = np.zeros((T, F, PW, PW), np.float32)
        xp[:, :, 1:65, 1:65] = x
        return np.ascontiguousarray(xp)

    def prep_gateW(W, bias):
        Wf = W[:, :, ::-1, :] if half else W
        Wp = np.ascontiguousarray(Wf[perm])  # [256, 72, 3, 3]
        bp = bias[perm].astype(np.float32)
        # x part: rows (ky*3+kx)*8+ic
        lx = Wp[:, :F].transpose(2, 3, 1, 0).reshape(72, 256)
        lp = [np.concatenate([Wp[:, F:, k, 0].T, Wp[:, F:, k, 2].T], axis=0)
              for k in range(3)]  # [128, 256]
        ls = [np.ascontiguousarray(Wp[:, F:, k, 1].T) for k in range(3)]
        return (np.ascontiguousarray(lx),
                [np.ascontiguousarray(a) for a in lp], ls,
                np.ascontiguousarray(bp[0:128].reshape(128, 1)),
                np.ascontiguousarray(bp[128:256].reshape(128, 1)))

    ex, ep, es, eb0, eb1 = prep_gateW(enc_W, enc_b)
    dx, dp, ds, db0, db1 = prep_gateW(dec_W, dec_b)
    oWf = out_W[:, :, ::-1, :] if half else out_W
    op = [np.ascontiguousarray(np.concatenate(
        [oWf[:, :, k, 0].T, oWf[:, :, k, 2].T], axis=0).astype(np.float32))
        for k in range(3)]
    osng = [np.ascontiguousarray(oWf[:, :, k, 1].T.astype(np.float32))
            for k in range(3)]

    m = {"xe": prep_x(enc_in), "xd": prep_x(dec_in),
         "w_ex": ex, "w_dx": dx,
         "b_e0": eb0, "b_e1": eb1, "b_d0": db0, "b_d1": db1,
         "b_o": np.ascontiguousarray(out_b.reshape(8, 1).astype(np.float32)),
         "zz": np.zeros((128, PW * PW), np.float32)}
    for k in range(3):
        m[f"w_ep{k}"] = ep[k]
        m[f"w_es{k}"] = es[k]
        m[f"w_dp{k}"] = dp[k]
        m[f"w_ds{k}"] = ds[k]
        m[f"w_op{k}"] = op[k]
        m[f"w_os{k}"] = osng[k]
    f32_keys = {"b_e0", "b_e1", "b_d0", "b_d1", "b_o"}
    return {k: np.ascontiguousarray(np.asarray(
        v, np.float32 if k in f32_keys else mm_np)) for k, v in m.items()}


def _install_trace_hook():
    """Shim antenv.axon_hooks for NTFF profiling (dev only)."""
    import contextlib
    import ctypes
    import types

    so = "/opt/axon/libaxon_pjrt.so"
    if "antenv.axon_hooks" in sys.modules or not os.path.exists(so):
        return
    lib = ctypes.CDLL(so)
    if not hasattr(lib, "axon_start_nrt_profile"):
        return
    lib.axon_start_nrt_profile.argtypes = [ctypes.POINTER(ctypes.c_int64),
                                           ctypes.c_size_t]
    lib.axon_start_nrt_profile.restype = ctypes.c_int64
    lib.axon_stop_nrt_profile.argtypes = [ctypes.c_char_p]
    lib.axon_stop_nrt_profile.restype = ctypes.c_int64

    def _mk():
        @contextlib.contextmanager
        def _hook(output_dir, device_ids):
            import jax
            jax.devices()
            if device_ids:
                ids = (ctypes.c_int64 * len(device_ids))(*device_ids)
                rc = lib.axon_start_nrt_profile(ids, len(device_ids))
            else:
                rc = lib.axon_start_nrt_profile(None, 0)
            if rc != 0:
                raise RuntimeError(f"axon_start_nrt_profile rc={rc}")
            try:
                yield
            finally:
                lib.axon_stop_nrt_profile(str(output_dir).encode())
        return _hook

    mod = types.ModuleType("antenv.axon_hooks")
    mod.get_axon_ntff_profile_hook = _mk
    sys.modules["antenv.axon_hooks"] = mod


def kernel(enc_in, dec_in, enc_W, enc_b, dec_W, dec_b, out_W, out_b):
    from concourse.bass_utils import run_bass_kernel_spmd

    trace = os.environ.get("KERNEL_TRACE", "") == "1"
    if trace:
        _install_trace_hook()

    use_bf16 = os.environ.get("KERNEL_DTYPE", "bf16") != "f32r"
    if "nc" not in _CACHE:
        _CACHE["nc"] = _build_program(use_bf16)
    nc = _CACHE["nc"]

    args = (np.asarray(enc_in, np.float32), np.asarray(dec_in, np.float32),
            np.asarray(enc_W, np.float32), np.asarray(enc_b, np.float32),
            np.asarray(dec_W, np.float32), np.asarray(dec_b, np.float32),
            np.asarray(out_W, np.float32), np.asarray(out_b, np.float32))
    in_maps = [_prep_core_inputs(c, *args, use_bf16=use_bf16)
               for c in range(NCORES)]

    res = run_bass_kernel_spmd(nc, in_maps, list(range(NCORES)), trace=trace)
    if trace:
        _CACHE["exec_time_ns"] = res.exec_time_ns

    B = enc_in.shape[0]
    out = np.empty((B, T, F, HS, WS), np.float32)
    for c in range(NCORES):
        b, half = c // 2, c % 2
        yc = res.results[c]["y"]  # [T, F, 32, 64]
        if half:
            out[b, :, :, 32:64, :] = yc[:, :, ::-1, :]
        else:
            out[b, :, :, 0:32, :] = yc
    return out


# revision 22
# speedup vs baseline: 1.2799x; 1.2568x over previous
"""EncDec ConvLSTM kernel for 8 Trainium2 NeuronCores.

Sharding: 8 cores = 4 (batch) x 2 (spatial row-halves). Each core computes
its 32 output rows plus a shrinking redundant halo (21-s extra rows at
recurrent step s), so no cross-core communication is needed. Row-half 1
cores receive a vertically flipped image and ky-flipped conv weights, so a
single SPMD program serves all cores.

Conv3x3 is mapped to PE matmuls over pixels (N=512 free dim, fp32r):
per 8-row tile the 4H=256 gate channels come from 2 M-tiles x 7
accumulating matmuls (1 x-im2col K=72 + 3 paired h-taps K=128 + 3 single
h-taps K=64). The kx=0/kx=2 h-taps are packed into one K=128 matmul using
a column-shifted copy of h kept in partitions 64..127.
"""

import os
import sys

import numpy as np

for _p in ("/opt/trn_rl_repo", "/root/.axon_site/_ro/trn_rl_repo"):
    if os.path.isdir(_p) and _p not in sys.path:
        sys.path.append(_p)

T = 10
F = 8
HD = 64
HS = 64
WS = 64
NCORES = 8
PW = 66  # padded grid width/height
NSTEPS = 2 * T

_CACHE = {}


def _regions():
    """Rounded compute-region row counts per recurrent step s=1..NSTEPS."""
    out = []
    for s in range(1, NSTEPS + 1):
        need = NSTEPS + 1 - s
        rows = min(HS, 32 + need)
        rows = min(HS, ((rows + 7) // 8) * 8)
        out.append(rows)
    return out


def _build_program(use_bf16=True):
    from concourse import bacc, mybir, tile

    F32 = mybir.dt.float32
    MMDT = mybir.dt.bfloat16 if use_bf16 else mybir.dt.float32r
    ACT = mybir.ActivationFunctionType

    nc = bacc.Bacc("TRN2", target_bir_lowering=False, debug=False,
                   num_devices=NCORES)

    def din(name, shape, dt=MMDT):
        return nc.dram_tensor(name, shape, dt, kind="ExternalInput").ap()

    xe_d = din("xe", [T, F, PW, PW])
    xd_d = din("xd", [T, F, PW, PW])
    w_x = {"e": din("w_ex", [72, 256]), "d": din("w_dx", [72, 256])}
    w_p = {ph: [din(f"w_{ph}p{k}", [128, 256]) for k in range(3)]
           for ph in ("e", "d")}
    w_s = {ph: [din(f"w_{ph}s{k}", [64, 256]) for k in range(3)]
           for ph in ("e", "d")}
    w_op = [din(f"w_op{k}", [128, 8]) for k in range(3)]
    w_os = [din(f"w_os{k}", [64, 8]) for k in range(3)]
    b_m0 = {"e": din("b_e0", [128, 1], F32), "d": din("b_d0", [128, 1], F32)}
    b_m1 = {"e": din("b_e1", [128, 1], F32), "d": din("b_d1", [128, 1], F32)}
    b_o = din("b_o", [8, 1], F32)
    zz_d = din("zz", [128, PW * PW])  # fp32r zeros for state init
    y_d = nc.dram_tensor("y", [T, F, 32, WS], F32, kind="ExternalOutput").ap()

    regions = _regions()

    with tile.TileContext(nc) as tc:
        with tc.tile_pool(name="wpool", bufs=1) as wp, \
             tc.tile_pool(name="state", bufs=1) as stp, \
             tc.tile_pool(name="x2p", bufs=2) as x2p, \
             tc.tile_pool(name="gps", bufs=6, space="PSUM") as gps, \
             tc.tile_pool(name="ops", bufs=2, space="PSUM") as ops, \
             tc.tile_pool(name="fip", bufs=3) as fip, \
             tc.tile_pool(name="ogp", bufs=3) as ogp, \
             tc.tile_pool(name="t1p", bufs=3) as t1p, \
             tc.tile_pool(name="t1lp", bufs=3) as t1lp, \
             tc.tile_pool(name="thp", bufs=3) as thp, \
             tc.tile_pool(name="yyp", bufs=2) as yyp:

            # ---- load weights / biases into SBUF ----
            def wtile(src, shape, tag, dt=MMDT):
                t_ = wp.tile(shape, dt, tag=tag)
                nc.sync.dma_start(t_[:], src[:])
                return t_

            sw_x = {ph: wtile(w_x[ph], [72, 256], f"wx{ph}")
                    for ph in ("e", "d")}
            sw_p = {ph: [wtile(w_p[ph][k], [128, 256], f"wp{ph}{k}")
                         for k in range(3)] for ph in ("e", "d")}
            sw_s = {ph: [wtile(w_s[ph][k], [64, 256], f"ws{ph}{k}")
                         for k in range(3)] for ph in ("e", "d")}
            sw_op = [wtile(w_op[k], [128, 8], f"wop{k}") for k in range(3)]
            sw_os = [wtile(w_os[k], [64, 8], f"wos{k}") for k in range(3)]
            sb_m0 = {ph: wtile(b_m0[ph], [128, 1], f"b0{ph}", F32)
                     for ph in ("e", "d")}
            sb_m1 = {ph: wtile(b_m1[ph], [128, 1], f"b1{ph}", F32)
                     for ph in ("e", "d")}
            sb_o = wtile(b_o, [8, 1], "bo", F32)

            # ---- persistent state ----
            hhA = stp.tile([128, PW * PW], MMDT, tag="hhA")
            hhB = stp.tile([128, PW * PW], MMDT, tag="hhB")
            c_t = stp.tile([64, PW * PW], F32, tag="c")
            nc.sync.dma_start(hhA[:], zz_d[:])
            nc.sync.dma_start(hhB[:], zz_d[:])
            nc.vector.memset(c_t[:], 0.0)

            # PE clock warm-up: a dense run of small-weight matmuls keeps
            # the PE array near-100% active so HAM raises the clock to
            # 2.4GHz before the real work starts. Gate matmuls alone never
            # warm it (128-col LDWEIGHTS between every MM lowers the
            # array's duty cycle below HAM's busy threshold).
            for _ in range(64):
                wu = ops.tile([8, 512], F32, tag="pso")
                nc.tensor.matmul(wu[:], sw_op[0][:], hhA[:, 0:512],
                                 start=True, stop=True)
            hhAv = hhA[:].rearrange("p (r c) -> p r c", c=PW)
            hhBv = hhB[:].rearrange("p (r c) -> p r c", c=PW)
            c_v = c_t[:].rearrange("p (r c) -> p r c", c=PW)

            def emit_x2col(s):
                """Load x im2col for step s: partition (ky*3+kx)*8+ic holds
                the flat padded image shifted by ky*66+kx (contiguous)."""
                ph = "e" if s <= T else "d"
                t_idx = (s - 1) if ph == "e" else (s - 1 - T)
                x_src = xe_d if ph == "e" else xd_d
                rp = regions[s - 1]
                ln = (rp - 1) * PW + 64
                x2 = x2p.tile([72, 57 * PW], MMDT, tag="x2")
                flat = x_src[t_idx].rearrange("a r c -> a (r c)")
                for tap in range(9):
                    sh = (tap // 3) * PW + (tap % 3)
                    nc.gpsimd.dma_start(x2[tap * 8:(tap + 1) * 8, 0:ln],
                                        flat[:, sh:sh + ln])
                return x2

            def emit_outconv(s, h_view):
                """relu(out conv) for decoder step s, reading its h buffer."""
                t_o = s - 1 - T
                for n2 in range(4):
                    r0 = n2 * 8
                    pso = ops.tile([8, 512], F32, tag="pso")
                    for k in range(3):
                        nc.tensor.matmul(pso[:], sw_op[k][:],
                                         h_view[:, r0 + k:r0 + k + 8, 0:64],
                                         start=(k == 0), stop=False)
                    for k in range(3):
                        nc.tensor.matmul(pso[:], sw_os[k][:],
                                         h_view[0:64, r0 + k:r0 + k + 8, 1:65],
                                         start=False, stop=(k == 2))
                    yy = yyp.tile([8, 512], F32, tag="yy")
                    nc.scalar.activation(yy[:], pso[:], ACT.Relu,
                                         bias=sb_o[:])
                    nc.gpsimd.dma_start(
                        y_d[t_o, :, r0:r0 + 8, :],
                        yy[:].rearrange("p (r c) -> p r c", c=64))

            x2_cur = emit_x2col(1)
            for s in range(1, NSTEPS + 1):
                ph = "e" if s <= T else "d"
                rp = regions[s - 1]
                ntiles = rp // 8
                h_r = hhAv if (s % 2 == 0) else hhBv  # read: written at s-1
                h_w = hhBv if (s % 2 == 0) else hhAv

                if s > T + 1:
                    emit_outconv(s - 1, h_r)  # prev decoder step, deps long resolved
                x2v = x2_cur[:].rearrange("p (r c) -> p r c", c=PW)
                if s < NSTEPS:
                    x2_next = emit_x2col(s + 1)  # prefetch on gpsimd queue

                if 1 < s <= T:
                    # re-warm burst: dense small-weight matmuls (8-col
                    # LDWEIGHTS) keep/restore the PE clock; the gate matmul
                    # pattern alone (128-col LDWEIGHTS per MM) falls under
                    # HAM's busy threshold. Decoder steps get this for free
                    # from the out-conv blocks. Reads x2 (read-only here).
                    for _ in range(10):
                        wu = ops.tile([8, 512], F32, tag="pso")
                        nc.tensor.matmul(wu[:], sw_os[0][:],
                                         x2v[0:64, 0:8, 0:64],
                                         start=True, stop=True)

                for n in range(ntiles):
                    r0 = n * 8
                    ps0 = gps.tile([128, 512], F32, tag="ps")
                    ps1 = gps.tile([128, 512], F32, tag="ps")
                    for m, ps in ((0, ps0), (1, ps1)):
                        ms = slice(m * 128, (m + 1) * 128)
                        nc.tensor.matmul(ps[:], sw_x[ph][:, ms],
                                         x2v[0:72, r0:r0 + 8, 0:64],
                                         start=True, stop=False)
                        for k in range(3):
                            nc.tensor.matmul(
                                ps[:], sw_p[ph][k][:, ms],
                                h_r[:, r0 + k:r0 + k + 8, 0:64],
                                start=False, stop=False)
                        for k in range(3):
                            nc.tensor.matmul(
                                ps[:], sw_s[ph][k][:, ms],
                                h_r[0:64, r0 + k:r0 + k + 8, 1:65],
                                start=False, stop=(k == 2))

                    # epilogue: M0=[f;i] M1=[o;g]
                    fi = fip.tile([128, 512], F32, tag="fi")
                    og = ogp.tile([128, 512], F32, tag="og")
                    nc.scalar.activation(fi[:], ps0[:], ACT.Sigmoid,
                                         bias=sb_m0[ph][:])
                    nc.scalar.activation(og[0:64], ps1[0:64], ACT.Sigmoid,
                                         bias=sb_m1[ph][0:64])
                    nc.scalar.activation(og[64:128], ps1[64:128], ACT.Tanh,
                                         bias=sb_m1[ph][64:128])
                    # t1 = sigmoid(i) * tanh(g) on partitions 64..127
                    t1 = t1p.tile([128, 512], F32, tag="t1")
                    nc.vector.tensor_mul(t1[64:128], fi[64:128], og[64:128])
                    # cross-partition move 64..127 -> 0..63
                    t1l = t1lp.tile([64, 512], F32, tag="t1l")
                    nc.sync.dma_start(t1l[:], t1[64:128])
                    t1lv = t1l[:].rearrange("p (r c) -> p r c", c=64)
                    cs = c_v[0:64, r0 + 1:r0 + 9, 1:65]
                    nc.vector.tensor_mul(cs, cs, fi[0:64].rearrange(
                        "p (r c) -> p r c", c=64))
                    nc.vector.tensor_add(cs, cs, t1lv)
                    th = thp.tile([64, 512], F32, tag="th")
                    thv = th[:].rearrange("p (r c) -> p r c", c=64)
                    nc.scalar.activation(thv, cs, ACT.Tanh)
                    # h = tanh(c) * sigmoid(o) -> base half of write buffer
                    nc.vector.tensor_mul(
                        h_w[0:64, r0 + 1:r0 + 9, 1:65], thv,
                        og[0:64].rearrange("p (r c) -> p r c", c=64))
                    # shifted copy (cols +2) into partitions 64..127
                    nc.sync.dma_start(
                        h_w[64:128, r0 + 1:r0 + 9, 0:64],
                        h_w[0:64, r0 + 1:r0 + 9, 2:66])

                if s < NSTEPS:
                    x2_cur = x2_next

            # out conv for the final decoder step
            emit_outconv(NSTEPS, hhBv if NSTEPS % 2 == 0 else hhAv)

    nc.compile()
    return nc


def _prep_core_inputs(core, enc_in, dec_in, enc_W, enc_b, dec_W, dec_b,
                      out_W, out_b, use_bf16=True):
    import ml_dtypes
    mm_np = ml_dtypes.bfloat16 if use_bf16 else np.float32
    b, half = core // 2, core % 2
    # gate permutation: [f, i, o, g]
    perm = np.concatenate([np.arange(0, 128), np.arange(192, 256),
                           np.arange(128, 192)])

    def prep_x(x):
        x = x[b]  # [T, F, 64, 64]
        if half:
            x = x[:, :, ::-1, :]
        xp = np.zeros((T, F, PW, PW), np.float32)
        xp[:, :, 1:65, 1:65] = x
        return np.ascontiguousarray(xp)

    def prep_gateW(W, bias):
        Wf = W[:, :, ::-1, :] if half else W
        Wp = np.ascontiguousarray(Wf[perm])  # [256, 72, 3, 3]
        bp = bias[perm].astype(np.float32)
        # x part: rows (ky*3+kx)*8+ic
        lx = Wp[:, :F].transpose(2, 3, 1, 0).reshape(72, 256)
        lp = [np.concatenate([Wp[:, F:, k, 0].T, Wp[:, F:, k, 2].T], axis=0)
              for k in range(3)]  # [128, 256]
        ls = [np.ascontiguousarray(Wp[:, F:, k, 1].T) for k in range(3)]
        return (np.ascontiguousarray(lx),
                [np.ascontiguousarray(a) for a in lp], ls,
                np.ascontiguousarray(bp[0:128].reshape(128, 1)),
                np.ascontiguousarray(bp[128:256].reshape(128, 1)))

    ex, ep, es, eb0, eb1 = prep_gateW(enc_W, enc_b)
    dx, dp, ds, db0, db1 = prep_gateW(dec_W, dec_b)
    oWf = out_W[:, :, ::-1, :] if half else out_W
    op = [np.ascontiguousarray(np.concatenate(
        [oWf[:, :, k, 0].T, oWf[:, :, k, 2].T], axis=0).astype(np.float32))
        for k in range(3)]
    osng = [np.ascontiguousarray(oWf[:, :, k, 1].T.astype(np.float32))
            for k in range(3)]

    m = {"xe": prep_x(enc_in), "xd": prep_x(dec_in),
         "w_ex": ex, "w_dx": dx,
         "b_e0": eb0, "b_e1": eb1, "b_d0": db0, "b_d1": db1,
         "b_o": np.ascontiguousarray(out_b.reshape(8, 1).astype(np.float32)),
         "zz": np.zeros((128, PW * PW), np.float32)}
    for k in range(3):
        m[f"w_ep{k}"] = ep[k]
        m[f"w_es{k}"] = es[k]
        m[f"w_dp{k}"] = dp[k]
        m[f"w_ds{k}"] = ds[k]
        m[f"w_op{k}"] = op[k]
        m[f"w_os{k}"] = osng[k]
    f32_keys = {"b_e0", "b_e1", "b_d0", "b_d1", "b_o"}
    return {k: np.ascontiguousarray(np.asarray(
        v, np.float32 if k in f32_keys else mm_np)) for k, v in m.items()}


def _install_trace_hook():
    """Shim antenv.axon_hooks for NTFF profiling (dev only)."""
    import contextlib
    import ctypes
    import types

    so = "/opt/axon/libaxon_pjrt.so"
    if "antenv.axon_hooks" in sys.modules or not os.path.exists(so):
        return
    lib = ctypes.CDLL(so)
    if not hasattr(lib, "axon_start_nrt_profile"):
        return
    lib.axon_start_nrt_profile.argtypes = [ctypes.POINTER(ctypes.c_int64),
                                           ctypes.c_size_t]
    lib.axon_start_nrt_profile.restype = ctypes.c_int64
    lib.axon_stop_nrt_profile.argtypes = [ctypes.c_char_p]
    lib.axon_stop_nrt_profile.restype = ctypes.c_int64

    def _mk():
        @contextlib.contextmanager
        def _hook(output_dir, device_ids):
            import jax
            jax.devices()
            if device_ids:
                ids = (ctypes.c_int64 * len(device_ids))(*device_ids)
                rc = lib.axon_start_nrt_profile(ids, len(device_ids))
            else:
                rc = lib.axon_start_nrt_profile(None, 0)
            if rc != 0:
                raise RuntimeError(f"axon_start_nrt_profile rc={rc}")
            try:
                yield
            finally:
                lib.axon_stop_nrt_profile(str(output_dir).encode())
        return _hook

    mod = types.ModuleType("antenv.axon_hooks")
    mod.get_axon_ntff_profile_hook = _mk
    sys.modules["antenv.axon_hooks"] = mod


def kernel(enc_in, dec_in, enc_W, enc_b, dec_W, dec_b, out_W, out_b):
    from concourse.bass_utils import run_bass_kernel_spmd

    trace = os.environ.get("KERNEL_TRACE", "") == "1"
    if trace:
        _install_trace_hook()

    use_bf16 = os.environ.get("KERNEL_DTYPE", "bf16") != "f32r"
    if "nc" not in _CACHE:
        _CACHE["nc"] = _build_program(use_bf16)
    nc = _CACHE["nc"]

    args = (np.asarray(enc_in, np.float32), np.asarray(dec_in, np.float32),
            np.asarray(enc_W, np.float32), np.asarray(enc_b, np.float32),
            np.asarray(dec_W, np.float32), np.asarray(dec_b, np.float32),
            np.asarray(out_W, np.float32), np.asarray(out_b, np.float32))
    in_maps = [_prep_core_inputs(c, *args, use_bf16=use_bf16)
               for c in range(NCORES)]

    res = run_bass_kernel_spmd(nc, in_maps, list(range(NCORES)), trace=trace)
    if trace:
        _CACHE["exec_time_ns"] = res.exec_time_ns

    B = enc_in.shape[0]
    out = np.empty((B, T, F, HS, WS), np.float32)
    for c in range(NCORES):
        b, half = c // 2, c % 2
        yc = res.results[c]["y"]  # [T, F, 32, 64]
        if half:
            out[b, :, :, 32:64, :] = yc[:, :, ::-1, :]
        else:
            out[b, :, :, 0:32, :] = yc
    return out
